# revision 4
# baseline (speedup 1.0000x reference)
"""MoE routing kernel for Trainium2 (8 NeuronCores).

Math (per reference):
  S = sigmoid(x @ Wg^T); top-2 gates G at indices I; w[t,e] = G if selected else 0
  down = sum_e w[:,e] * (x @ Wd[e]^T)          # [T, Dg]
  up   = sum_e w[:,e] * (down @ Wu[e]^T)       # [T, D]

Strategy: data-parallel over tokens — each of the 8 cores handles T/8 = 512
tokens and computes all 8 experts densely (top-2 applied via gate weights).
Host passes x^T / Wg^T / per-expert-transposed Wd, Wu so every on-chip matmul
has its contraction dim on partitions with zero large on-chip transposes.

Per-core dataflow (tokens token-major, 4 tiles of 128):
  gate:  ST[e,t] = Wg^T-chunks (lhsT) x xT-chunks (rhs, fp32) accumulated in
         PSUM; PE-transpose 128-token slices -> Z[t,e]; top-2 via two
         reduce_max passes; w = sigmoid(Z) * (Z >= second_max)
  down:  P_pair[t, 2*Dg] += xT-chunk (lhsT, bf16) x WdT-pair-chunk (rhs)
         over 16 K-chunks; combine dacc = sum_e w_e * P_e on DVE
  Z:     Z_e = w_e * dacc (ACT per-partition scale, bf16); PE-transpose to
         ZT_e[g, t]
  up:    U[t, dblk] += ZT_e-chunk (lhsT) x WuT-chunk (rhs) accumulated over
         (e, g-chunk) in PSUM; copy to SBUF; DMA out token-major.
"""

import numpy as np
import ml_dtypes

import concourse.bass as bass
import concourse.mybir as mybir
import concourse.tile as tile
from concourse.bacc import Bacc
from concourse.bass_utils import run_bass_kernel_spmd

BF16 = mybir.dt.bfloat16
F32 = mybir.dt.float32
AF = mybir.ActivationFunctionType
ALU = mybir.AluOpType
AX = mybir.AxisListType

NCORES = 8
B, L, D, E, DG = 2, 2048, 2048, 8, 256
T = B * L            # 4096 tokens
TC = T // NCORES     # 512 tokens per core
P = 128
NDC = D // P         # 16 contraction chunks over D
NTT = TC // P        # 4 token tiles per core
DBLK = 512           # free-dim block for the up matmul
NDB = D // DBLK      # 4
NPAIR = E // 2       # 4 expert pairs (2 experts share one PSUM bank)
NGC = DG // P        # 2 contraction chunks over Dg


def build_moe(nc: bass.Bass):
    xT = nc.dram_tensor("xT", [D, TC], F32, kind="ExternalInput")
    WgT = nc.dram_tensor("WgT", [D, E], F32, kind="ExternalInput")
    Wdp = nc.dram_tensor("Wdp", [NPAIR, D, 2 * DG], BF16, kind="ExternalInput")
    WuTt = nc.dram_tensor("WuTt", [E, DG, D], BF16, kind="ExternalInput")
    idb = nc.dram_tensor("idb", [P, P], BF16, kind="ExternalInput")
    idf = nc.dram_tensor("idf", [P, P], F32, kind="ExternalInput")
    out = nc.dram_tensor("out", [TC, D], F32, kind="ExternalOutput")

    with tile.TileContext(nc) as tc:
        with (
            tc.tile_pool(name="res", bufs=1) as res,
            tc.tile_pool(name="stream", bufs=3) as stream,
            tc.tile_pool(name="small", bufs=2) as small,
            tc.tile_pool(name="ps", bufs=1, space="PSUM") as ps,
        ):
            # ---------- constants / resident tiles ----------
            ident_b = res.tile([P, P], BF16, tag="identb", name="ident_b")
            nc.sync.dma_start(ident_b[:], idb[:, :])
            ident_f = res.tile([P, P], F32, tag="identf", name="ident_f")
            nc.sync.dma_start(ident_f[:], idf[:, :])

            wg_sb = res.tile([P, NDC, E], F32, tag="wg", name="wg_sb")
            for dc in range(NDC):
                nc.sync.dma_start(wg_sb[:, dc, :], WgT[dc * P : (dc + 1) * P, :])

            xbf = res.tile([P, NDC, TC], BF16, tag="xbf", name="xbf")

            # ---------- stream x^T chunks: cast to bf16 + gate matmul ----------
            st_ps = ps.tile([E, TC], F32, tag="bank", bufs=5, name="st_ps")
            for dc in range(NDC):
                # bufs=NDC: each chunk gets its own slot so the load DMA never
                # carries WAR waits (HWDGE descriptors cap sync-wait count)
                xt = stream.tile([P, TC], F32, tag="xt", bufs=NDC, name=f"xt{dc}")
                nc.sync.dma_start(xt[:], xT[dc * P : (dc + 1) * P, :])
                nc.scalar.copy(xbf[:, dc, :], xt[:])
                nc.tensor.matmul(
                    st_ps[:],
                    wg_sb[:, dc, :],
                    xt[:],
                    start=(dc == 0),
                    stop=(dc == NDC - 1),
                )

            # ---------- expert weight loads (overlap with compute) ----------
            wd_sb = []
            for pr in range(NPAIR):
                t = res.tile([P, NDC, 2 * DG], BF16, tag=f"wd{pr}", name=f"wd{pr}")
                nc.sync.dma_start(t[:], Wdp[pr].rearrange("(dc p) g -> p dc g", p=P))
                wd_sb.append(t)
            wu_sb = []
            for e in range(E):
                t = res.tile([P, NGC, D], BF16, tag=f"wu{e}", name=f"wu{e}")
                nc.sync.dma_start(t[:], WuTt[e].rearrange("(gc p) d -> p gc d", p=P))
                wu_sb.append(t)

            # ---------- gate: transpose to token-major, top-2, weights ----------
            st_sb = res.tile([E, TC], F32, tag="stsb", name="st_sb")
            nc.vector.tensor_copy(st_sb[:], st_ps[:])
            w_tiles = []
            for tt in range(NTT):
                ztok = ps.tile([P, E], F32, tag="ztok", bufs=1, name=f"ztok{tt}")
                nc.tensor.transpose(
                    ztok[:], st_sb[:, tt * P : (tt + 1) * P], ident_f[:E, :E]
                )
                m1 = small.tile([P, 1], F32, tag="m1", name=f"m1_{tt}")
                nc.vector.reduce_max(m1[:], ztok[:], axis=AX.X)
                # tmp = Z + (Z == m1) * -1e30  (mask out the max)
                tmp = small.tile([P, E], F32, tag="tmp", name=f"tmp{tt}")
                nc.vector.tensor_scalar(
                    tmp[:], ztok[:], m1[:], -1e30, ALU.is_equal, ALU.mult
                )
                nc.vector.tensor_tensor(tmp[:], tmp[:], ztok[:], ALU.add)
                m2 = small.tile([P, 1], F32, tag="m2", name=f"m2_{tt}")
                nc.vector.reduce_max(m2[:], tmp[:], axis=AX.X)
                g = small.tile([P, E], F32, tag="g", name=f"g{tt}")
                nc.scalar.activation(g[:], ztok[:], AF.Sigmoid)
                msk = small.tile([P, E], F32, tag="msk", name=f"msk{tt}")
                nc.vector.tensor_scalar(msk[:], ztok[:], m2[:], None, ALU.is_ge)
                w = res.tile([P, E], F32, tag=f"w{tt}", name=f"w{tt}")
                nc.vector.tensor_tensor(w[:], g[:], msk[:], ALU.mult)
                w_tiles.append(w)

            # ---------- down projection + gate combine + Z^T ----------
            # zt_sb[g, e, gc, t] holds (w_e * down)^T ready as lhsT for up
            zt_sb = res.tile([P, E, NGC, TC], BF16, tag="zt", name="zt_sb")
            for tt in range(NTT):
                ts_ = slice(tt * P, (tt + 1) * P)
                ppair = []
                for pr in range(NPAIR):
                    pt = ps.tile(
                        [P, 2 * DG], F32, tag="bank", bufs=5, name=f"pd{tt}_{pr}"
                    )
                    ppair.append(pt)
                for dc in range(NDC):
                    for pr in range(NPAIR):
                        nc.tensor.matmul(
                            ppair[pr][:],
                            xbf[:, dc, ts_],
                            wd_sb[pr][:, dc, :],
                            start=(dc == 0),
                            stop=(dc == NDC - 1),
                        )
                dacc = stream.tile([P, DG], F32, tag="dacc", name=f"dacc{tt}")
                nc.vector.tensor_scalar(
                    dacc[:], ppair[0][:, 0:DG], w_tiles[tt][:, 0:1], None, ALU.mult
                )
                for e in range(1, E):
                    nc.vector.scalar_tensor_tensor(
                        dacc[:],
                        ppair[e // 2][:, (e % 2) * DG : (e % 2 + 1) * DG],
                        w_tiles[tt][:, e : e + 1],
                        dacc[:],
                        ALU.mult,
                        ALU.add,
                    )
                for e in range(E):
                    z = stream.tile([P, DG], BF16, tag="z", name=f"z{tt}_{e}")
                    nc.scalar.activation(
                        z[:], dacc[:], AF.Copy, scale=w_tiles[tt][:, e : e + 1]
                    )
                    for gc in range(NGC):
                        ztr = ps.tile(
                            [P, P], BF16, tag="tr", bufs=2, name=f"tr{tt}_{e}_{gc}"
                        )
                        nc.tensor.transpose(
                            ztr[:], z[:, gc * P : (gc + 1) * P], ident_b[:]
                        )
                        nc.vector.tensor_copy(zt_sb[:, e, gc, ts_], ztr[:])

            # ---------- up projection ----------
            for tt in range(NTT):
                ts_ = slice(tt * P, (tt + 1) * P)
                for db in range(NDB):
                    u = ps.tile([P, DBLK], F32, tag="bank", bufs=5, name=f"u{tt}_{db}")
                    kk = 0
                    for e in range(E):
                        for gc in range(NGC):
                            nc.tensor.matmul(
                                u[:],
                                zt_sb[:, e, gc, ts_],
                                wu_sb[e][:, gc, db * DBLK : (db + 1) * DBLK],
                                start=(kk == 0),
                                stop=(kk == E * NGC - 1),
                            )
                            kk += 1
                    usb = stream.tile([P, DBLK], F32, tag="usb", name=f"usb{tt}_{db}")
                    nc.scalar.copy(usb[:], u[:])
                    nc.sync.dma_start(
                        out[tt * P : (tt + 1) * P, db * DBLK : (db + 1) * DBLK], usb[:]
                    )
    return nc


_CACHE = {}


def get_nc() -> bass.Bass:
    if "nc" not in _CACHE:
        nc = Bacc()
        build_moe(nc)
        # bacc compile splits multi-sem waits into EventSemaphores (walrus
        # accepts at most one embedded wait per instruction) among other
        # legalization passes.
        nc.compile()
        _CACHE["nc"] = nc
    return _CACHE["nc"]


def prep_in_maps(x, Wg, Wd, Wu):
    bf = ml_dtypes.bfloat16
    xf = np.asarray(x, np.float32).reshape(T, D)
    xTf = np.ascontiguousarray(xf.T)                       # [D, T]
    WgTh = np.ascontiguousarray(np.asarray(Wg, np.float32).T)  # [D, E]
    WdT = np.asarray(Wd, np.float32).transpose(0, 2, 1)    # [E, D, DG]
    # pair p holds experts (2p, 2p+1) side by side on the free dim
    Wdp_h = np.ascontiguousarray(
        np.concatenate([WdT[0::2], WdT[1::2]], axis=2)
    ).astype(bf)                                           # [NPAIR, D, 2*DG]
    WuT_h = np.ascontiguousarray(
        np.asarray(Wu, np.float32).transpose(0, 2, 1)
    ).astype(bf)                                           # [E, DG, D]
    idb = np.eye(P, dtype=bf)
    idf = np.eye(P, dtype=np.float32)
    shared = dict(WgT=WgTh, Wdp=Wdp_h, WuTt=WuT_h, idb=idb, idf=idf)
    in_maps = []
    for c in range(NCORES):
        m = dict(shared)
        m["xT"] = np.ascontiguousarray(xTf[:, c * TC : (c + 1) * TC])
        in_maps.append(m)
    return in_maps


def kernel(x, Wg, Wd, Wu, k):
    assert int(k) == 2, f"kernel hardcodes top-2 routing, got k={k}"
    nc = get_nc()
    in_maps = prep_in_maps(x, Wg, Wd, Wu)
    res = run_bass_kernel_spmd(nc, in_maps, core_ids=list(range(NCORES)))
    outs = [res.results[c]["out"] for c in range(NCORES)]
    return np.ascontiguousarray(
        np.concatenate(outs, axis=0).reshape(B, L, D), dtype=np.float32
    )
